# revision 8
# baseline (speedup 1.0000x reference)
"""ButterflyConv Trainium2 kernel.

Reference computation (per batch image):
  now = x
  for s in 0..5:
    left  = leaky(dwconv3x3(now,           W[2s])   + b[2s])
    right = leaky(dwconv3x3(now[masks[s]], W[2s+1]) + b[2s+1])
    now = left + right
  out = now + x
with leaky = LeakyReLU(0.05), SAME padding, depthwise (per-channel) 3x3 convs.

Mapping (per NeuronCore, 2 of 16 batch images):
  - 128 SBUF partitions = (batch 2) x (channel 64).
  - Free dim = image rows, padded to 194 cols (1 zero col each side) so
    horizontal conv shifts are plain element offsets.
  - H=192 in 2 bands of 96 output rows with a 6-row halo (6 chained 3x3
    stages shrink the valid region 1 row/stage).
  - conv-R (gathered branch, 9 taps) + 4 taps of conv-L run on the
    TensorEngine as PSUM-accumulated [128,128] matmuls whose lhsT holds
    W[conv, dst_c, dy, dx] at (src_p, dst_p); the channel gather is
    folded into the matrices. ~0.42ns/elem/tap.
  - Remaining 5 conv-L taps on VectorE as tensor_scalar (4x) +
    tensor_tensor (2x) pairs with per-partition weight scalars.
  - leaky(v+b) is decomposed as relu(0.95v + 0.95b) + (0.05v + 0.05b):
    both pieces are single ScalarE activation passes (Relu / Identity
    with per-partition bias, reading PSUM directly).
  - Combine adds are split across VectorE and GpSimd.
Data is bf16 on-chip; matmul accumulation is fp32 in PSUM; the final
residual add is computed in fp32.
"""

import numpy as np
import ml_dtypes

C = 64
H = 192
W_IMG = 192
NB = 6
BC = 2          # batch per core
P = 128
NCORES = 16 // BC
WPAD = W_IMG + 2
NEG = 0.05

# tap order: (dy, dx); first 9 = conv-R on PE; next 4 = conv-L PE taps
TAPS = [(dy, dx) for dy in range(3) for dx in range(3)]
L_PE_TAPS = [5, 6, 7, 8]          # conv-L taps done on PE
L_DVE_TAPS = [0, 1, 2, 3, 4]      # conv-L taps done on DVE
NMAT = 9 + len(L_PE_TAPS)         # matrices per stage
CH_ROWS = 10                      # epilogue chunk rows
CH_N = CH_ROWS * WPAD             # 1940 elems, fits 4 PSUM banks (2048)

_PROG_CACHE = {}


def _build_program(h=H, r_band=96):
    import concourse.bacc as bacc
    import concourse.mybir as mybir
    from concourse.tile import TileContext
    from concourse.alu_op_type import AluOpType

    f32 = mybir.dt.float32
    bf16 = mybir.dt.bfloat16
    ident = mybir.ActivationFunctionType.Identity
    relu = mybir.ActivationFunctionType.Relu

    S = r_band + 12               # band span rows
    SZ = S * WPAD + 2             # tile free size (+2 guard elems)
    n_bands = (h + r_band - 1) // r_band

    nc = bacc.Bacc("TRN2", target_bir_lowering=False, debug=False,
                   enable_asserts=False, num_devices=1)

    xs_d = nc.dram_tensor("xs", [P, h * W_IMG], f32, kind="ExternalInput").ap()
    rmat_d = nc.dram_tensor("rmat", [P, NB * NMAT * P], bf16,
                            kind="ExternalInput").ap()
    wsc_d = nc.dram_tensor("wsc", [P, NB * 9], f32, kind="ExternalInput").ap()
    bl_d = nc.dram_tensor("bl", [P, NB], f32, kind="ExternalInput").ap()
    # right-branch biases pre-scaled by 0.95 / 0.05 on host
    br95_d = nc.dram_tensor("br95", [P, NB], f32, kind="ExternalInput").ap()
    br05_d = nc.dram_tensor("br05", [P, NB], f32, kind="ExternalInput").ap()
    out_d = nc.dram_tensor("out", [P, h * W_IMG], f32, kind="ExternalOutput").ap()

    with TileContext(nc) as tc:
        with tc.tile_pool(name="big", bufs=1) as bigp, \
             tc.tile_pool(name="tab", bufs=1) as tabp, \
             tc.tile_pool(name="scr", bufs=2) as scrp, \
             tc.tile_pool(name="stg", bufs=3) as stgp, \
             tc.tile_pool(name="psR", bufs=1, space="PSUM") as pspR, \
             tc.tile_pool(name="psL", bufs=1, space="PSUM") as pspL:

            # --- static tables ---
            rmat_sb = tabp.tile([P, NB * NMAT * P], bf16, tag="rmat")
            nc.sync.dma_start(out=rmat_sb[:], in_=rmat_d)
            wsc_sb = tabp.tile([P, NB * 9], f32, tag="wsc")
            nc.sync.dma_start(out=wsc_sb[:], in_=wsc_d)
            bl_sb = tabp.tile([P, NB], f32, tag="bl")
            nc.sync.dma_start(out=bl_sb[:], in_=bl_d)
            br95_sb = tabp.tile([P, NB], f32, tag="br95")
            nc.sync.dma_start(out=br95_sb[:], in_=br95_d)
            br05_sb = tabp.tile([P, NB], f32, tag="br05")
            nc.sync.dma_start(out=br05_sb[:], in_=br05_d)

            # --- persistent image buffers ---
            A = bigp.tile([P, SZ], bf16, tag="A")   # band of x (residual source)
            B = bigp.tile([P, SZ], bf16, tag="B")
            D = bigp.tile([P, SZ], bf16, tag="D")
            for t in (A, B, D):
                nc.gpsimd.memset(t[:], 0.0)

            def v2d(t, row0, nrows, col0, ncols):
                return (t[:, 1:1 + S * WPAD]
                        .rearrange("p (r w) -> p r w", w=WPAD)
                        [:, row0:row0 + nrows, col0:col0 + ncols])

            def lhsT(s, t):
                i = (s * NMAT + t) * P
                return rmat_sb[:, i:i + P]

            for bi in range(n_bands):
                out_lo = bi * r_band
                out_hi = min(h, out_lo + r_band)
                in_lo = out_lo - 6

                # zero halo rows (6 top + 6 bottom of span)
                for t in (A, B, D):
                    nc.gpsimd.memset(t[:, 1:1 + 6 * WPAD], 0.0)
                    nc.gpsimd.memset(t[:, 1 + (S - 6) * WPAD:1 + S * WPAD], 0.0)

                # load x band (f32 -> bf16 cast DMA)
                img_lo = max(0, in_lo)
                img_hi = min(h, in_lo + S)
                nrows_ld = img_hi - img_lo
                r0 = img_lo - in_lo
                n_dma = 4
                step = (nrows_ld + n_dma - 1) // n_dma
                for k in range(0, nrows_ld, step):
                    kk = min(step, nrows_ld - k)
                    src = (xs_d[:, (img_lo + k) * W_IMG:(img_lo + k + kk) * W_IMG]
                           .rearrange("p (r w) -> p r w", w=W_IMG))
                    nc.gpsimd.dma_start(out=v2d(A, r0 + k, kk, 1, W_IMG), in_=src)

                xbuf = [A, B, D, B, D, B]
                ybuf = [B, D, B, D, B, D]
                for s in range(NB):
                    X, Y = xbuf[s], ybuf[s]
                    c_lo = max(0, out_lo - 5 + s)
                    c_hi = min(h, out_hi + 5 - s)
                    nr = c_hi - c_lo
                    r_off = c_lo - in_lo
                    e0 = 1 + r_off * WPAD

                    for cr0 in range(0, nr, CH_ROWS):
                        cr = min(CH_ROWS, nr - cr0)
                        n = cr * WPAD
                        base = e0 + cr0 * WPAD
                        # sub-chunks of <=512 elems, PSUM-bank aligned
                        subs = []
                        so = 0
                        while so < n:
                            subs.append((so, min(512, n - so)))
                            so += 512

                        # ---- conv-R: 9 PE taps -> psumR ----
                        psR = pspR.tile([P, 2048], f32, tag="psR")
                        for so, sn in subs:
                            for t in range(9):
                                dy, dx = TAPS[t]
                                off = (dy - 1) * WPAD + (dx - 1)
                                nc.tensor.matmul(
                                    psR[:, so:so + sn], lhsT(s, t),
                                    X[:, base + off + so:base + off + so + sn],
                                    start=(t == 0), stop=(t == 8))
                        # rR = 0.95*relu(vR+bR), uR = 0.05*(vR+bR)
                        rR = scrp.tile([P, CH_N], bf16, tag="rR")
                        nc.scalar.activation(
                            out=rR[:, :n], in_=psR[:, :n], func=relu,
                            bias=br95_sb[:, s:s + 1], scale=0.95)
                        uR = scrp.tile([P, CH_N], bf16, tag="uR")
                        nc.scalar.activation(
                            out=uR[:, :n], in_=psR[:, :n], func=ident,
                            bias=br05_sb[:, s:s + 1], scale=0.05)

                        # ---- conv-L: 4 PE taps -> psumL ----
                        psL = pspL.tile([P, 2048], f32, tag="psL")
                        for so, sn in subs:
                            for j, t in enumerate(L_PE_TAPS):
                                dy, dx = TAPS[t]
                                off = (dy - 1) * WPAD + (dx - 1)
                                nc.tensor.matmul(
                                    psL[:, so:so + sn], lhsT(s, 9 + j),
                                    X[:, base + off + so:base + off + so + sn],
                                    start=(j == 0), stop=(j == len(L_PE_TAPS) - 1))
                        cL = scrp.tile([P, CH_N], bf16, tag="cL")
                        nc.scalar.activation(
                            out=cL[:, :n], in_=psL[:, :n], func=ident,
                            bias=0.0, scale=0.05)

                        # ---- conv-L: 5 DVE taps into Y chunk (+bias) ----
                        ysl = v2d(Y, r_off + cr0, cr, 1, W_IMG)

                        def xv(t):
                            dy, dx = TAPS[t]
                            return v2d(X, r_off + cr0 + dy - 1, cr, dx, W_IMG)

                        t0 = L_DVE_TAPS[0]
                        nc.vector.tensor_scalar(
                            out=ysl, in0=xv(t0),
                            scalar1=wsc_sb[:, s * 9 + t0:s * 9 + t0 + 1],
                            scalar2=bl_sb[:, s:s + 1],
                            op0=AluOpType.mult, op1=AluOpType.add)
                        for t in L_DVE_TAPS[1:]:
                            tmp = scrp.tile([P, CH_N], bf16, tag="tmp")
                            tv = (tmp[:, :n].rearrange("p (r w) -> p r w", w=WPAD)
                                  [:, :, 1:1 + W_IMG])
                            nc.vector.tensor_scalar(
                                out=tv, in0=xv(t),
                                scalar1=wsc_sb[:, s * 9 + t:s * 9 + t + 1],
                                scalar2=0.0,
                                op0=AluOpType.mult, op1=AluOpType.add)
                            nc.vector.tensor_tensor(
                                out=ysl, in0=ysl, in1=tv, op=AluOpType.add)
                        # += PE part of conv-L
                        cLv = (cL[:, :n].rearrange("p (r w) -> p r w", w=WPAD)
                               [:, :, 1:1 + W_IMG])
                        nc.gpsimd.tensor_tensor(
                            out=ysl, in0=ysl, in1=cLv, op=AluOpType.add)

                        # ---- leaky-L + combine ----
                        # rL = 0.95*relu(accL) (ACT), Y = 0.05*accL + rL + rR + uR
                        rL = scrp.tile([P, CH_N], bf16, tag="rL")
                        rLv = (rL[:, :n].rearrange("p (r w) -> p r w", w=WPAD)
                               [:, :, 1:1 + W_IMG])
                        nc.scalar.activation(
                            out=rLv, in_=ysl, func=relu, bias=0.0, scale=19.0)
                        nc.vector.tensor_tensor(
                            out=ysl, in0=ysl, in1=rLv, op=AluOpType.add)
                        rRv = (rR[:, :n].rearrange("p (r w) -> p r w", w=WPAD)
                               [:, :, 1:1 + W_IMG])
                        nc.vector.tensor_tensor(
                            out=ysl, in0=ysl, in1=rRv, op=AluOpType.add)
                        uRv = (uR[:, :n].rearrange("p (r w) -> p r w", w=WPAD)
                               [:, :, 1:1 + W_IMG])
                        nc.gpsimd.tensor_tensor(
                            out=ysl, in0=ysl, in1=uRv, op=AluOpType.add)

                # ---- residual + store (T6 = ybuf[5] = D) ----
                rows_per = 8
                for k in range(out_lo, out_hi, rows_per):
                    kk = min(rows_per, out_hi - k)
                    st = stgp.tile([P, rows_per * W_IMG], f32, tag="st")
                    stv = (st[:, :kk * W_IMG]
                           .rearrange("p (r w) -> p r w", w=W_IMG))
                    nc.gpsimd.tensor_tensor(
                        out=stv, in0=v2d(D, k - in_lo, kk, 1, W_IMG),
                        in1=v2d(A, k - in_lo, kk, 1, W_IMG), op=AluOpType.add)
                    nc.sync.dma_start(
                        out=out_d[:, k * W_IMG:(k + kk) * W_IMG],
                        in_=st[:, :kk * W_IMG])

    nc.compile()
    return nc


def _host_tables(W, b, masks):
    """Build device-layout weight tables from full inputs."""
    Wt = np.asarray(W, np.float32).reshape(2 * NB, C, 3, 3)
    bt = np.asarray(b, np.float32)
    masks = np.asarray(masks, np.int64)

    # PE tap matrices: [P, NB*NMAT*P] bf16; lhsT[src, dst] per (stage, tap)
    rmat = np.zeros((NB, NMAT, P, P), np.float32)
    dst_c = np.arange(C)
    for s in range(NB):
        src_c = masks[s]
        for t in range(9):      # conv-R taps (gathered)
            dy, dx = TAPS[t]
            vals = Wt[2 * s + 1, dst_c, dy, dx]
            for bb in range(BC):
                rmat[s, t, bb * C + src_c, bb * C + dst_c] = vals
        for j, t in enumerate(L_PE_TAPS):   # conv-L PE taps (identity)
            dy, dx = TAPS[t]
            vals = Wt[2 * s, dst_c, dy, dx]
            for bb in range(BC):
                rmat[s, 9 + j, bb * C + dst_c, bb * C + dst_c] = vals
    rmat_sb = np.ascontiguousarray(
        rmat.transpose(2, 0, 1, 3).reshape(P, NB * NMAT * P))
    rmat_sb = rmat_sb.astype(ml_dtypes.bfloat16)

    # conv-L DVE weights and bias pre-scaled by 0.05: accL is accumulated
    # as 0.05*accL so leaky(accL) = relu(19*accL') + accL' needs no extra op
    pc = np.tile(np.arange(C), BC)
    wsc = np.zeros((P, NB * 9), np.float32)
    bl = np.zeros((P, NB), np.float32)
    br = np.zeros((P, NB), np.float32)
    for s in range(NB):
        for t in range(9):
            wsc[:, s * 9 + t] = 0.05 * Wt[2 * s, pc, t // 3, t % 3]
        bl[:, s] = 0.05 * bt[2 * s, pc]
        br[:, s] = bt[2 * s + 1, pc]
    return rmat_sb, wsc, bl, 0.95 * br, 0.05 * br


def _get_prog():
    key = (H, 96)
    if key not in _PROG_CACHE:
        _PROG_CACHE[key] = _build_program(H, 96)
    return _PROG_CACHE[key]


def _run_on_hw(nc, in_maps, trace=False, **kw):
    from concourse import bass_utils
    return bass_utils.run_bass_kernel_spmd(
        nc, in_maps, core_ids=list(range(len(in_maps))), trace=trace, **kw)


def _make_in_maps(x, W, b, masks):
    rmat_sb, wsc, bl, br95, br05 = _host_tables(W, b, masks)
    x = np.asarray(x, np.float32)
    nb_total = x.shape[0]
    in_maps = []
    for k in range(0, nb_total, BC):
        xs = np.ascontiguousarray(x[k:k + BC].reshape(BC * C, H * W_IMG))
        in_maps.append({"xs": xs, "rmat": rmat_sb, "wsc": wsc,
                       "bl": bl, "br95": br95, "br05": br05})
    return in_maps


def kernel(x, W, b, masks):
    nc = _get_prog()
    in_maps = _make_in_maps(x, W, b, masks)
    res = _run_on_hw(nc, in_maps)
    outs = [r["out"].reshape(BC, C, H, W_IMG).astype(np.float32)
            for r in res.results]
    return np.concatenate(outs, axis=0)


# revision 9
# speedup vs baseline: 1.0132x; 1.0132x over previous
"""ButterflyConv Trainium2 kernel.

Reference computation (per batch image):
  now = x
  for s in 0..5:
    left  = leaky(dwconv3x3(now,           W[2s])   + b[2s])
    right = leaky(dwconv3x3(now[masks[s]], W[2s+1]) + b[2s+1])
    now = left + right
  out = now + x
with leaky = LeakyReLU(0.05), SAME padding, depthwise (per-channel) 3x3 convs.

Mapping (per NeuronCore, 2 of 16 batch images):
  - 128 SBUF partitions = (batch 2) x (channel 64).
  - Free dim = image rows, padded to 194 cols (1 zero col each side) so
    horizontal conv shifts are plain element offsets.
  - H=192 in 2 bands of 96 output rows with a 6-row halo (6 chained 3x3
    stages shrink the valid region 1 row/stage).
  - conv-R (gathered branch, 9 taps) + 4 taps of conv-L run on the
    TensorEngine as PSUM-accumulated [128,128] matmuls whose lhsT holds
    W[conv, dst_c, dy, dx] at (src_p, dst_p); the channel gather is
    folded into the matrices. ~0.42ns/elem/tap.
  - Remaining 5 conv-L taps on VectorE as tensor_scalar (4x) +
    tensor_tensor (2x) pairs with per-partition weight scalars.
  - leaky(v+b) is decomposed as relu(0.95v + 0.95b) + (0.05v + 0.05b):
    both pieces are single ScalarE activation passes (Relu / Identity
    with per-partition bias, reading PSUM directly).
  - Combine adds are split across VectorE and GpSimd.
Data is bf16 on-chip; matmul accumulation is fp32 in PSUM; the final
residual add is computed in fp32.
"""

import numpy as np
import ml_dtypes

C = 64
H = 192
W_IMG = 192
NB = 6
BC = 2          # batch per core
P = 128
NCORES = 16 // BC
WPAD = W_IMG + 2
NEG = 0.05

# tap order: (dy, dx); first 9 = conv-R on PE; next 4 = conv-L PE taps
TAPS = [(dy, dx) for dy in range(3) for dx in range(3)]
L_PE_TAPS = [5, 6, 7, 8]          # conv-L taps done on PE
L_DVE_TAPS = [0, 1, 2, 3, 4]      # conv-L taps done on DVE
NMAT = 9 + len(L_PE_TAPS)         # matrices per stage
CH_ROWS = 10                      # epilogue chunk rows
CH_N = CH_ROWS * WPAD             # 1940 elems, fits 4 PSUM banks (2048)

_PROG_CACHE = {}


def _build_program(h=H, r_band=96):
    import concourse.bacc as bacc
    import concourse.mybir as mybir
    from concourse.tile import TileContext
    from concourse.alu_op_type import AluOpType

    f32 = mybir.dt.float32
    bf16 = mybir.dt.bfloat16
    ident = mybir.ActivationFunctionType.Identity
    relu = mybir.ActivationFunctionType.Relu

    S = r_band + 12               # band span rows
    SZ = S * WPAD + 2             # tile free size (+2 guard elems)
    n_bands = (h + r_band - 1) // r_band

    nc = bacc.Bacc("TRN2", target_bir_lowering=False, debug=False,
                   enable_asserts=False, num_devices=1)

    xs_d = nc.dram_tensor("xs", [P, h * W_IMG], f32, kind="ExternalInput").ap()
    rmat_d = nc.dram_tensor("rmat", [P, NB * NMAT * P], bf16,
                            kind="ExternalInput").ap()
    wsc_d = nc.dram_tensor("wsc", [P, NB * 9], f32, kind="ExternalInput").ap()
    bl_d = nc.dram_tensor("bl", [P, NB], f32, kind="ExternalInput").ap()
    # right-branch biases pre-scaled by 0.95 / 0.05 on host
    br95_d = nc.dram_tensor("br95", [P, NB], f32, kind="ExternalInput").ap()
    br05_d = nc.dram_tensor("br05", [P, NB], f32, kind="ExternalInput").ap()
    out_d = nc.dram_tensor("out", [P, h * W_IMG], f32, kind="ExternalOutput").ap()

    with TileContext(nc) as tc:
        with tc.tile_pool(name="big", bufs=1) as bigp, \
             tc.tile_pool(name="tab", bufs=1) as tabp, \
             tc.tile_pool(name="scr", bufs=3) as scrp, \
             tc.tile_pool(name="stg", bufs=3) as stgp, \
             tc.tile_pool(name="psR", bufs=1, space="PSUM") as pspR, \
             tc.tile_pool(name="psL", bufs=1, space="PSUM") as pspL:

            # --- static tables ---
            rmat_sb = tabp.tile([P, NB * NMAT * P], bf16, tag="rmat")
            nc.sync.dma_start(out=rmat_sb[:], in_=rmat_d)
            wsc_sb = tabp.tile([P, NB * 9], f32, tag="wsc")
            nc.sync.dma_start(out=wsc_sb[:], in_=wsc_d)
            bl_sb = tabp.tile([P, NB], f32, tag="bl")
            nc.sync.dma_start(out=bl_sb[:], in_=bl_d)
            br95_sb = tabp.tile([P, NB], f32, tag="br95")
            nc.sync.dma_start(out=br95_sb[:], in_=br95_d)
            br05_sb = tabp.tile([P, NB], f32, tag="br05")
            nc.sync.dma_start(out=br05_sb[:], in_=br05_d)

            # --- persistent image buffers ---
            A = bigp.tile([P, SZ], bf16, tag="A")   # band of x (residual source)
            B = bigp.tile([P, SZ], bf16, tag="B")
            D = bigp.tile([P, SZ], bf16, tag="D")
            for t in (A, B, D):
                nc.gpsimd.memset(t[:], 0.0)

            def v2d(t, row0, nrows, col0, ncols):
                return (t[:, 1:1 + S * WPAD]
                        .rearrange("p (r w) -> p r w", w=WPAD)
                        [:, row0:row0 + nrows, col0:col0 + ncols])

            def lhsT(s, t):
                i = (s * NMAT + t) * P
                return rmat_sb[:, i:i + P]

            for bi in range(n_bands):
                out_lo = bi * r_band
                out_hi = min(h, out_lo + r_band)
                in_lo = out_lo - 6

                # zero halo rows (6 top + 6 bottom of span)
                for t in (A, B, D):
                    nc.gpsimd.memset(t[:, 1:1 + 6 * WPAD], 0.0)
                    nc.gpsimd.memset(t[:, 1 + (S - 6) * WPAD:1 + S * WPAD], 0.0)

                # load x band (f32 -> bf16 cast DMA)
                img_lo = max(0, in_lo)
                img_hi = min(h, in_lo + S)
                nrows_ld = img_hi - img_lo
                r0 = img_lo - in_lo
                n_dma = 4
                step = (nrows_ld + n_dma - 1) // n_dma
                for k in range(0, nrows_ld, step):
                    kk = min(step, nrows_ld - k)
                    src = (xs_d[:, (img_lo + k) * W_IMG:(img_lo + k + kk) * W_IMG]
                           .rearrange("p (r w) -> p r w", w=W_IMG))
                    nc.gpsimd.dma_start(out=v2d(A, r0 + k, kk, 1, W_IMG), in_=src)

                xbuf = [A, B, D, B, D, B]
                ybuf = [B, D, B, D, B, D]
                for s in range(NB):
                    X, Y = xbuf[s], ybuf[s]
                    c_lo = max(0, out_lo - 5 + s)
                    c_hi = min(h, out_hi + 5 - s)
                    nr = c_hi - c_lo
                    r_off = c_lo - in_lo
                    e0 = 1 + r_off * WPAD

                    for cr0 in range(0, nr, CH_ROWS):
                        cr = min(CH_ROWS, nr - cr0)
                        n = cr * WPAD
                        base = e0 + cr0 * WPAD
                        # sub-chunks of <=512 elems, PSUM-bank aligned
                        subs = []
                        so = 0
                        while so < n:
                            subs.append((so, min(512, n - so)))
                            so += 512

                        # ---- conv-R: 9 PE taps -> psumR ----
                        psR = pspR.tile([P, 2048], f32, tag="psR")
                        for so, sn in subs:
                            for t in range(9):
                                dy, dx = TAPS[t]
                                off = (dy - 1) * WPAD + (dx - 1)
                                nc.tensor.matmul(
                                    psR[:, so:so + sn], lhsT(s, t),
                                    X[:, base + off + so:base + off + so + sn],
                                    start=(t == 0), stop=(t == 8))
                        # rR = 0.95*relu(vR+bR), uR = 0.05*(vR+bR)
                        rR = scrp.tile([P, CH_N], bf16, tag="rR")
                        nc.scalar.activation(
                            out=rR[:, :n], in_=psR[:, :n], func=relu,
                            bias=br95_sb[:, s:s + 1], scale=0.95)
                        uR = scrp.tile([P, CH_N], bf16, tag="uR")
                        nc.scalar.activation(
                            out=uR[:, :n], in_=psR[:, :n], func=ident,
                            bias=br05_sb[:, s:s + 1], scale=0.05)

                        # ---- conv-L: 4 PE taps -> psumL ----
                        psL = pspL.tile([P, 2048], f32, tag="psL")
                        for so, sn in subs:
                            for j, t in enumerate(L_PE_TAPS):
                                dy, dx = TAPS[t]
                                off = (dy - 1) * WPAD + (dx - 1)
                                nc.tensor.matmul(
                                    psL[:, so:so + sn], lhsT(s, 9 + j),
                                    X[:, base + off + so:base + off + so + sn],
                                    start=(j == 0), stop=(j == len(L_PE_TAPS) - 1))
                        cL = scrp.tile([P, CH_N], bf16, tag="cL")
                        nc.scalar.activation(
                            out=cL[:, :n], in_=psL[:, :n], func=ident,
                            bias=0.0, scale=0.05)

                        # ---- conv-L: 5 DVE taps into Y chunk (+bias) ----
                        ysl = v2d(Y, r_off + cr0, cr, 1, W_IMG)

                        def xv(t):
                            dy, dx = TAPS[t]
                            return v2d(X, r_off + cr0 + dy - 1, cr, dx, W_IMG)

                        t0 = L_DVE_TAPS[0]
                        nc.vector.tensor_scalar(
                            out=ysl, in0=xv(t0),
                            scalar1=wsc_sb[:, s * 9 + t0:s * 9 + t0 + 1],
                            scalar2=bl_sb[:, s:s + 1],
                            op0=AluOpType.mult, op1=AluOpType.add)
                        for t in L_DVE_TAPS[1:]:
                            tmp = scrp.tile([P, CH_N], bf16, tag="rL")
                            tv = (tmp[:, :n].rearrange("p (r w) -> p r w", w=WPAD)
                                  [:, :, 1:1 + W_IMG])
                            nc.vector.tensor_scalar(
                                out=tv, in0=xv(t),
                                scalar1=wsc_sb[:, s * 9 + t:s * 9 + t + 1],
                                scalar2=0.0,
                                op0=AluOpType.mult, op1=AluOpType.add)
                            nc.vector.tensor_tensor(
                                out=ysl, in0=ysl, in1=tv, op=AluOpType.add)
                        # += PE part of conv-L
                        cLv = (cL[:, :n].rearrange("p (r w) -> p r w", w=WPAD)
                               [:, :, 1:1 + W_IMG])
                        nc.gpsimd.tensor_tensor(
                            out=ysl, in0=ysl, in1=cLv, op=AluOpType.add)

                        # ---- leaky-L + combine ----
                        # rL = 0.95*relu(accL) (ACT), Y = 0.05*accL + rL + rR + uR
                        rL = scrp.tile([P, CH_N], bf16, tag="rL")
                        rLv = (rL[:, :n].rearrange("p (r w) -> p r w", w=WPAD)
                               [:, :, 1:1 + W_IMG])
                        nc.scalar.activation(
                            out=rLv, in_=ysl, func=relu, bias=0.0, scale=19.0)
                        nc.vector.tensor_tensor(
                            out=ysl, in0=ysl, in1=rLv, op=AluOpType.add)
                        rRv = (rR[:, :n].rearrange("p (r w) -> p r w", w=WPAD)
                               [:, :, 1:1 + W_IMG])
                        nc.vector.tensor_tensor(
                            out=ysl, in0=ysl, in1=rRv, op=AluOpType.add)
                        uRv = (uR[:, :n].rearrange("p (r w) -> p r w", w=WPAD)
                               [:, :, 1:1 + W_IMG])
                        nc.gpsimd.tensor_tensor(
                            out=ysl, in0=ysl, in1=uRv, op=AluOpType.add)

                # ---- residual + store (T6 = ybuf[5] = D) ----
                rows_per = 4
                for k in range(out_lo, out_hi, rows_per):
                    kk = min(rows_per, out_hi - k)
                    st = stgp.tile([P, rows_per * W_IMG], f32, tag="st")
                    stv = (st[:, :kk * W_IMG]
                           .rearrange("p (r w) -> p r w", w=W_IMG))
                    nc.gpsimd.tensor_tensor(
                        out=stv, in0=v2d(D, k - in_lo, kk, 1, W_IMG),
                        in1=v2d(A, k - in_lo, kk, 1, W_IMG), op=AluOpType.add)
                    nc.sync.dma_start(
                        out=out_d[:, k * W_IMG:(k + kk) * W_IMG],
                        in_=st[:, :kk * W_IMG])

    nc.compile()
    return nc


def _host_tables(W, b, masks):
    """Build device-layout weight tables from full inputs."""
    Wt = np.asarray(W, np.float32).reshape(2 * NB, C, 3, 3)
    bt = np.asarray(b, np.float32)
    masks = np.asarray(masks, np.int64)

    # PE tap matrices: [P, NB*NMAT*P] bf16; lhsT[src, dst] per (stage, tap)
    rmat = np.zeros((NB, NMAT, P, P), np.float32)
    dst_c = np.arange(C)
    for s in range(NB):
        src_c = masks[s]
        for t in range(9):      # conv-R taps (gathered)
            dy, dx = TAPS[t]
            vals = Wt[2 * s + 1, dst_c, dy, dx]
            for bb in range(BC):
                rmat[s, t, bb * C + src_c, bb * C + dst_c] = vals
        for j, t in enumerate(L_PE_TAPS):   # conv-L PE taps (identity)
            dy, dx = TAPS[t]
            vals = Wt[2 * s, dst_c, dy, dx]
            for bb in range(BC):
                rmat[s, 9 + j, bb * C + dst_c, bb * C + dst_c] = vals
    rmat_sb = np.ascontiguousarray(
        rmat.transpose(2, 0, 1, 3).reshape(P, NB * NMAT * P))
    rmat_sb = rmat_sb.astype(ml_dtypes.bfloat16)

    # conv-L DVE weights and bias pre-scaled by 0.05: accL is accumulated
    # as 0.05*accL so leaky(accL) = relu(19*accL') + accL' needs no extra op
    pc = np.tile(np.arange(C), BC)
    wsc = np.zeros((P, NB * 9), np.float32)
    bl = np.zeros((P, NB), np.float32)
    br = np.zeros((P, NB), np.float32)
    for s in range(NB):
        for t in range(9):
            wsc[:, s * 9 + t] = 0.05 * Wt[2 * s, pc, t // 3, t % 3]
        bl[:, s] = 0.05 * bt[2 * s, pc]
        br[:, s] = bt[2 * s + 1, pc]
    return rmat_sb, wsc, bl, 0.95 * br, 0.05 * br


def _get_prog():
    key = (H, 96)
    if key not in _PROG_CACHE:
        _PROG_CACHE[key] = _build_program(H, 96)
    return _PROG_CACHE[key]


def _run_on_hw(nc, in_maps, trace=False, **kw):
    from concourse import bass_utils
    return bass_utils.run_bass_kernel_spmd(
        nc, in_maps, core_ids=list(range(len(in_maps))), trace=trace, **kw)


def _make_in_maps(x, W, b, masks):
    rmat_sb, wsc, bl, br95, br05 = _host_tables(W, b, masks)
    x = np.asarray(x, np.float32)
    nb_total = x.shape[0]
    in_maps = []
    for k in range(0, nb_total, BC):
        xs = np.ascontiguousarray(x[k:k + BC].reshape(BC * C, H * W_IMG))
        in_maps.append({"xs": xs, "rmat": rmat_sb, "wsc": wsc,
                       "bl": bl, "br95": br95, "br05": br05})
    return in_maps


def kernel(x, W, b, masks):
    nc = _get_prog()
    in_maps = _make_in_maps(x, W, b, masks)
    res = _run_on_hw(nc, in_maps)
    outs = [r["out"].reshape(BC, C, H, W_IMG).astype(np.float32)
            for r in res.results]
    return np.concatenate(outs, axis=0)


# revision 10
# speedup vs baseline: 1.3548x; 1.3372x over previous
"""ButterflyConv Trainium2 kernel.

Reference computation (per batch image):
  now = x
  for s in 0..5:
    left  = leaky(dwconv3x3(now,           W[2s])   + b[2s])
    right = leaky(dwconv3x3(now[masks[s]], W[2s+1]) + b[2s+1])
    now = left + right
  out = now + x
with leaky = LeakyReLU(0.05), SAME padding, depthwise (per-channel) 3x3 convs.

Mapping (per NeuronCore, 2 of 16 batch images):
  - 128 SBUF partitions = (batch 2) x (channel 64).
  - Free dim = image rows, padded to 194 cols (1 zero col each side) so
    horizontal conv shifts are plain element offsets.
  - H=192 in 2 bands of 96 output rows with a 6-row halo (6 chained 3x3
    stages shrink the valid region 1 row/stage).
  - conv-R (gathered branch, 9 taps) + 4 taps of conv-L run on the
    TensorEngine as PSUM-accumulated [128,128] matmuls whose lhsT holds
    W[conv, dst_c, dy, dx] at (src_p, dst_p); the channel gather is
    folded into the matrices. ~0.42ns/elem/tap.
  - Remaining 5 conv-L taps on VectorE as tensor_scalar (4x) +
    tensor_tensor (2x) pairs with per-partition weight scalars.
  - leaky(v+b) is decomposed as relu(0.95v + 0.95b) + (0.05v + 0.05b):
    both pieces are single ScalarE activation passes (Relu / Identity
    with per-partition bias, reading PSUM directly).
  - Combine adds are split across VectorE and GpSimd.
Data is bf16 on-chip; matmul accumulation is fp32 in PSUM; the final
residual add is computed in fp32.
"""

import numpy as np
import ml_dtypes

C = 64
H = 192
W_IMG = 192
NB = 6
BC = 2          # batch per core
P = 128
NCORES = 16 // BC
WPAD = W_IMG + 2
NEG = 0.05

# tap order: (dy, dx); first 9 = conv-R on PE; next 4 = conv-L PE taps
TAPS = [(dy, dx) for dy in range(3) for dx in range(3)]
L_PE_TAPS = [5, 6, 7, 8]          # conv-L taps done on PE
L_DVE_TAPS = [0, 1, 2, 3, 4]      # conv-L taps done on DVE
NMAT = 9 + len(L_PE_TAPS)         # matrices per stage
CH_ROWS = 10                      # epilogue chunk rows
CH_N = CH_ROWS * WPAD             # 1940 elems, fits 4 PSUM banks (2048)

_PROG_CACHE = {}


def _build_program(h=H, r_band=96):
    import concourse.bacc as bacc
    import concourse.mybir as mybir
    from concourse.tile import TileContext
    from concourse.alu_op_type import AluOpType

    f32 = mybir.dt.float32
    bf16 = mybir.dt.bfloat16
    ident = mybir.ActivationFunctionType.Identity
    relu = mybir.ActivationFunctionType.Relu

    S = r_band + 12               # band span rows
    SZ = S * WPAD + 2             # tile free size (+2 guard elems)
    n_bands = (h + r_band - 1) // r_band

    nc = bacc.Bacc("TRN2", target_bir_lowering=False, debug=False,
                   enable_asserts=False, num_devices=1)

    xs_d = nc.dram_tensor("xs", [P, h * W_IMG], f32, kind="ExternalInput").ap()
    rmat_d = nc.dram_tensor("rmat", [P, NB * NMAT * P], bf16,
                            kind="ExternalInput").ap()
    wsc_d = nc.dram_tensor("wsc", [P, NB * 9], f32, kind="ExternalInput").ap()
    bl_d = nc.dram_tensor("bl", [P, NB], f32, kind="ExternalInput").ap()
    # right-branch biases pre-scaled by 0.95 / 0.05 on host
    br95_d = nc.dram_tensor("br95", [P, NB], f32, kind="ExternalInput").ap()
    br05_d = nc.dram_tensor("br05", [P, NB], f32, kind="ExternalInput").ap()
    out_d = nc.dram_tensor("out", [P, h * W_IMG], f32, kind="ExternalOutput").ap()

    with TileContext(nc) as tc:
        with tc.tile_pool(name="big", bufs=1) as bigp, \
             tc.tile_pool(name="tab", bufs=1) as tabp, \
             tc.tile_pool(name="scr", bufs=3) as scrp, \
             tc.tile_pool(name="stg", bufs=3) as stgp, \
             tc.tile_pool(name="psR", bufs=1, space="PSUM") as pspR, \
             tc.tile_pool(name="psL", bufs=1, space="PSUM") as pspL:

            # --- static tables ---
            rmat_sb = tabp.tile([P, NB * NMAT * P], bf16, tag="rmat")
            nc.sync.dma_start(out=rmat_sb[:], in_=rmat_d)
            wsc_sb = tabp.tile([P, NB * 9], f32, tag="wsc")
            nc.sync.dma_start(out=wsc_sb[:], in_=wsc_d)
            bl_sb = tabp.tile([P, NB], f32, tag="bl")
            nc.sync.dma_start(out=bl_sb[:], in_=bl_d)
            br95_sb = tabp.tile([P, NB], f32, tag="br95")
            nc.sync.dma_start(out=br95_sb[:], in_=br95_d)
            br05_sb = tabp.tile([P, NB], f32, tag="br05")
            nc.sync.dma_start(out=br05_sb[:], in_=br05_d)

            # --- persistent image buffers ---
            A = bigp.tile([P, SZ], bf16, tag="A")   # band of x (residual source)
            B = bigp.tile([P, SZ], bf16, tag="B")
            D = bigp.tile([P, SZ], bf16, tag="D")
            for t in (A, B, D):
                nc.gpsimd.memset(t[:], 0.0)

            def v2d(t, row0, nrows, col0, ncols):
                return (t[:, 1:1 + S * WPAD]
                        .rearrange("p (r w) -> p r w", w=WPAD)
                        [:, row0:row0 + nrows, col0:col0 + ncols])

            def lhsT(s, t):
                i = (s * NMAT + t) * P
                return rmat_sb[:, i:i + P]

            for bi in range(n_bands):
                out_lo = bi * r_band
                out_hi = min(h, out_lo + r_band)
                in_lo = out_lo - 6

                # zero halo rows (6 top + 6 bottom of span)
                for t in (A, B, D):
                    nc.gpsimd.memset(t[:, 1:1 + 6 * WPAD], 0.0)
                    nc.gpsimd.memset(t[:, 1 + (S - 6) * WPAD:1 + S * WPAD], 0.0)

                # load x band (f32 -> bf16 cast DMA)
                img_lo = max(0, in_lo)
                img_hi = min(h, in_lo + S)
                nrows_ld = img_hi - img_lo
                r0 = img_lo - in_lo
                n_dma = 4
                step = (nrows_ld + n_dma - 1) // n_dma
                for k in range(0, nrows_ld, step):
                    kk = min(step, nrows_ld - k)
                    src = (xs_d[:, (img_lo + k) * W_IMG:(img_lo + k + kk) * W_IMG]
                           .rearrange("p (r w) -> p r w", w=W_IMG))
                    nc.gpsimd.dma_start(out=v2d(A, r0 + k, kk, 1, W_IMG), in_=src)

                xbuf = [A, B, D, B, D, B]
                ybuf = [B, D, B, D, B, D]
                for s in range(NB):
                    X, Y = xbuf[s], ybuf[s]
                    c_lo = max(0, out_lo - 5 + s)
                    c_hi = min(h, out_hi + 5 - s)
                    nr = c_hi - c_lo
                    r_off = c_lo - in_lo
                    e0 = 1 + r_off * WPAD

                    for cr0 in range(0, nr, CH_ROWS):
                        cr = min(CH_ROWS, nr - cr0)
                        n = cr * WPAD
                        base = e0 + cr0 * WPAD
                        # sub-chunks of <=512 elems, PSUM-bank aligned
                        subs = []
                        so = 0
                        while so < n:
                            subs.append((so, min(512, n - so)))
                            so += 512

                        # ---- conv-R: 9 PE taps -> psumR ----
                        psR = pspR.tile([P, 2048], f32, tag="psR")
                        for so, sn in subs:
                            for t in range(9):
                                dy, dx = TAPS[t]
                                off = (dy - 1) * WPAD + (dx - 1)
                                nc.tensor.matmul(
                                    psR[:, so:so + sn], lhsT(s, t),
                                    X[:, base + off + so:base + off + so + sn],
                                    start=(t == 0), stop=(t == 8))
                        # rR = 0.95*relu(vR+bR), uR = 0.05*(vR+bR)
                        rR = scrp.tile([P, CH_N], bf16, tag="rR")
                        nc.scalar.activation(
                            out=rR[:, :n], in_=psR[:, :n], func=relu,
                            bias=br95_sb[:, s:s + 1], scale=0.95)
                        uR = scrp.tile([P, CH_N], bf16, tag="uR")
                        nc.scalar.activation(
                            out=uR[:, :n], in_=psR[:, :n], func=ident,
                            bias=br05_sb[:, s:s + 1], scale=0.05)

                        # ---- conv-L: 4 PE taps -> psumL ----
                        psL = pspL.tile([P, 2048], f32, tag="psL")
                        for so, sn in subs:
                            for j, t in enumerate(L_PE_TAPS):
                                dy, dx = TAPS[t]
                                off = (dy - 1) * WPAD + (dx - 1)
                                nc.tensor.matmul(
                                    psL[:, so:so + sn], lhsT(s, 9 + j),
                                    X[:, base + off + so:base + off + so + sn],
                                    start=(j == 0), stop=(j == len(L_PE_TAPS) - 1))
                        cL = scrp.tile([P, CH_N], bf16, tag="cL")
                        nc.scalar.activation(
                            out=cL[:, :n], in_=psL[:, :n], func=ident,
                            bias=0.0, scale=0.05)

                        # ---- conv-L: 5 DVE taps into Y chunk (+bias) ----
                        ysl = v2d(Y, r_off + cr0, cr, 1, W_IMG)

                        def xv(t):
                            dy, dx = TAPS[t]
                            return v2d(X, r_off + cr0 + dy - 1, cr, dx, W_IMG)

                        t0 = L_DVE_TAPS[0]
                        nc.vector.tensor_scalar(
                            out=ysl, in0=xv(t0),
                            scalar1=wsc_sb[:, s * 9 + t0:s * 9 + t0 + 1],
                            scalar2=bl_sb[:, s:s + 1],
                            op0=AluOpType.mult, op1=AluOpType.add)
                        for t in L_DVE_TAPS[1:]:
                            tmp = scrp.tile([P, CH_N], bf16, tag="rL")
                            tv = (tmp[:, :n].rearrange("p (r w) -> p r w", w=WPAD)
                                  [:, :, 1:1 + W_IMG])
                            nc.vector.tensor_scalar(
                                out=tv, in0=xv(t),
                                scalar1=wsc_sb[:, s * 9 + t:s * 9 + t + 1],
                                scalar2=0.0,
                                op0=AluOpType.mult, op1=AluOpType.add)
                            nc.vector.tensor_tensor(
                                out=ysl, in0=ysl, in1=tv, op=AluOpType.add)
                        # += PE part of conv-L
                        cLv = (cL[:, :n].rearrange("p (r w) -> p r w", w=WPAD)
                               [:, :, 1:1 + W_IMG])
                        nc.vector.tensor_tensor(
                            out=ysl, in0=ysl, in1=cLv, op=AluOpType.add)

                        # ---- leaky-L + combine ----
                        # rL = 0.95*relu(accL) (ACT), Y = 0.05*accL + rL + rR + uR
                        rL = scrp.tile([P, CH_N], bf16, tag="rL")
                        rLv = (rL[:, :n].rearrange("p (r w) -> p r w", w=WPAD)
                               [:, :, 1:1 + W_IMG])
                        nc.scalar.activation(
                            out=rLv, in_=ysl, func=relu, bias=0.0, scale=19.0)
                        nc.vector.tensor_tensor(
                            out=ysl, in0=ysl, in1=rLv, op=AluOpType.add)
                        rRv = (rR[:, :n].rearrange("p (r w) -> p r w", w=WPAD)
                               [:, :, 1:1 + W_IMG])
                        nc.vector.tensor_tensor(
                            out=ysl, in0=ysl, in1=rRv, op=AluOpType.add)
                        uRv = (uR[:, :n].rearrange("p (r w) -> p r w", w=WPAD)
                               [:, :, 1:1 + W_IMG])
                        nc.vector.tensor_tensor(
                            out=ysl, in0=ysl, in1=uRv, op=AluOpType.add)

                # ---- residual + store (T6 = ybuf[5] = D) ----
                rows_per = 4
                for k in range(out_lo, out_hi, rows_per):
                    kk = min(rows_per, out_hi - k)
                    st = stgp.tile([P, rows_per * W_IMG], f32, tag="st")
                    stv = (st[:, :kk * W_IMG]
                           .rearrange("p (r w) -> p r w", w=W_IMG))
                    nc.vector.tensor_tensor(
                        out=stv, in0=v2d(D, k - in_lo, kk, 1, W_IMG),
                        in1=v2d(A, k - in_lo, kk, 1, W_IMG), op=AluOpType.add)
                    nc.sync.dma_start(
                        out=out_d[:, k * W_IMG:(k + kk) * W_IMG],
                        in_=st[:, :kk * W_IMG])

    nc.compile()
    return nc


def _host_tables(W, b, masks):
    """Build device-layout weight tables from full inputs."""
    Wt = np.asarray(W, np.float32).reshape(2 * NB, C, 3, 3)
    bt = np.asarray(b, np.float32)
    masks = np.asarray(masks, np.int64)

    # PE tap matrices: [P, NB*NMAT*P] bf16; lhsT[src, dst] per (stage, tap)
    rmat = np.zeros((NB, NMAT, P, P), np.float32)
    dst_c = np.arange(C)
    for s in range(NB):
        src_c = masks[s]
        for t in range(9):      # conv-R taps (gathered)
            dy, dx = TAPS[t]
            vals = Wt[2 * s + 1, dst_c, dy, dx]
            for bb in range(BC):
                rmat[s, t, bb * C + src_c, bb * C + dst_c] = vals
        for j, t in enumerate(L_PE_TAPS):   # conv-L PE taps (identity)
            dy, dx = TAPS[t]
            vals = Wt[2 * s, dst_c, dy, dx]
            for bb in range(BC):
                rmat[s, 9 + j, bb * C + dst_c, bb * C + dst_c] = vals
    rmat_sb = np.ascontiguousarray(
        rmat.transpose(2, 0, 1, 3).reshape(P, NB * NMAT * P))
    rmat_sb = rmat_sb.astype(ml_dtypes.bfloat16)

    # conv-L DVE weights and bias pre-scaled by 0.05: accL is accumulated
    # as 0.05*accL so leaky(accL) = relu(19*accL') + accL' needs no extra op
    pc = np.tile(np.arange(C), BC)
    wsc = np.zeros((P, NB * 9), np.float32)
    bl = np.zeros((P, NB), np.float32)
    br = np.zeros((P, NB), np.float32)
    for s in range(NB):
        for t in range(9):
            wsc[:, s * 9 + t] = 0.05 * Wt[2 * s, pc, t // 3, t % 3]
        bl[:, s] = 0.05 * bt[2 * s, pc]
        br[:, s] = bt[2 * s + 1, pc]
    return rmat_sb, wsc, bl, 0.95 * br, 0.05 * br


def _get_prog():
    key = (H, 96)
    if key not in _PROG_CACHE:
        _PROG_CACHE[key] = _build_program(H, 96)
    return _PROG_CACHE[key]


def _run_on_hw(nc, in_maps, trace=False, **kw):
    from concourse import bass_utils
    return bass_utils.run_bass_kernel_spmd(
        nc, in_maps, core_ids=list(range(len(in_maps))), trace=trace, **kw)


def _make_in_maps(x, W, b, masks):
    rmat_sb, wsc, bl, br95, br05 = _host_tables(W, b, masks)
    x = np.asarray(x, np.float32)
    nb_total = x.shape[0]
    in_maps = []
    for k in range(0, nb_total, BC):
        xs = np.ascontiguousarray(x[k:k + BC].reshape(BC * C, H * W_IMG))
        in_maps.append({"xs": xs, "rmat": rmat_sb, "wsc": wsc,
                       "bl": bl, "br95": br95, "br05": br05})
    return in_maps


def kernel(x, W, b, masks):
    nc = _get_prog()
    in_maps = _make_in_maps(x, W, b, masks)
    res = _run_on_hw(nc, in_maps)
    outs = [r["out"].reshape(BC, C, H, W_IMG).astype(np.float32)
            for r in res.results]
    return np.concatenate(outs, axis=0)


# revision 11
# speedup vs baseline: 1.4912x; 1.1007x over previous
"""ButterflyConv Trainium2 kernel.

Reference computation (per batch image):
  now = x
  for s in 0..5:
    left  = leaky(dwconv3x3(now,           W[2s])   + b[2s])
    right = leaky(dwconv3x3(now[masks[s]], W[2s+1]) + b[2s+1])
    now = left + right
  out = now + x
with leaky = LeakyReLU(0.05), SAME padding, depthwise (per-channel) 3x3 convs.

Mapping (per NeuronCore, 2 of 16 batch images):
  - 128 SBUF partitions = (batch 2) x (channel 64).
  - Free dim = image rows, padded to 194 cols (1 zero col each side) so
    horizontal conv shifts are plain element offsets.
  - H=192 in 2 bands of 96 output rows with a 6-row halo (6 chained 3x3
    stages shrink the valid region 1 row/stage).
  - conv-R (gathered branch, 9 taps) + 4 taps of conv-L run on the
    TensorEngine as PSUM-accumulated [128,128] matmuls whose lhsT holds
    W[conv, dst_c, dy, dx] at (src_p, dst_p); the channel gather is
    folded into the matrices. ~0.42ns/elem/tap.
  - Remaining 5 conv-L taps on VectorE as tensor_scalar (4x) +
    tensor_tensor (2x) pairs with per-partition weight scalars.
  - leaky(v+b) is decomposed as relu(0.95v + 0.95b) + (0.05v + 0.05b):
    both pieces are single ScalarE activation passes (Relu / Identity
    with per-partition bias, reading PSUM directly).
  - Combine adds are split across VectorE and GpSimd.
Data is bf16 on-chip; matmul accumulation is fp32 in PSUM; the final
residual add is computed in fp32.
"""

import numpy as np
import ml_dtypes

C = 64
H = 192
W_IMG = 192
NB = 6
BC = 2          # batch per core
P = 128
NCORES = 16 // BC
WPAD = W_IMG + 2
NEG = 0.05

# tap order: (dy, dx); first 9 = conv-R on PE; next 4 = conv-L PE taps
TAPS = [(dy, dx) for dy in range(3) for dx in range(3)]
L_PE_TAPS = [4, 5, 6, 7, 8]       # conv-L taps done on PE
L_DVE_TAPS = [0, 1, 2, 3]         # conv-L taps done on DVE
NMAT = 9 + len(L_PE_TAPS)         # matrices per stage
CH_ROWS = 10                      # epilogue chunk rows
CH_N = CH_ROWS * WPAD             # 1940 elems, fits 4 PSUM banks (2048)

_PROG_CACHE = {}


def _build_program(h=H, r_band=96):
    import concourse.bacc as bacc
    import concourse.mybir as mybir
    from concourse.tile import TileContext
    from concourse.alu_op_type import AluOpType

    f32 = mybir.dt.float32
    bf16 = mybir.dt.bfloat16
    ident = mybir.ActivationFunctionType.Identity
    relu = mybir.ActivationFunctionType.Relu

    S = r_band + 12               # band span rows
    SZ = S * WPAD + 2             # tile free size (+2 guard elems)
    n_bands = (h + r_band - 1) // r_band

    nc = bacc.Bacc("TRN2", target_bir_lowering=False, debug=False,
                   enable_asserts=False, num_devices=1)

    xs_d = nc.dram_tensor("xs", [P, h * W_IMG], f32, kind="ExternalInput").ap()
    rmat_d = nc.dram_tensor("rmat", [P, NB * NMAT * P], bf16,
                            kind="ExternalInput").ap()
    wsc_d = nc.dram_tensor("wsc", [P, NB * 9], f32, kind="ExternalInput").ap()
    bl_d = nc.dram_tensor("bl", [P, NB], f32, kind="ExternalInput").ap()
    # right-branch biases pre-scaled by 0.95 / 0.05 on host
    br95_d = nc.dram_tensor("br95", [P, NB], f32, kind="ExternalInput").ap()
    br05_d = nc.dram_tensor("br05", [P, NB], f32, kind="ExternalInput").ap()
    out_d = nc.dram_tensor("out", [P, h * W_IMG], f32, kind="ExternalOutput").ap()

    with TileContext(nc) as tc:
        with tc.tile_pool(name="big", bufs=1) as bigp, \
             tc.tile_pool(name="tab", bufs=1) as tabp, \
             tc.tile_pool(name="scr", bufs=3) as scrp, \
             tc.tile_pool(name="stg", bufs=3) as stgp, \
             tc.tile_pool(name="psR", bufs=1, space="PSUM") as pspR, \
             tc.tile_pool(name="psL", bufs=1, space="PSUM") as pspL:

            # --- static tables ---
            rmat_sb = tabp.tile([P, NB * NMAT * P], bf16, tag="rmat")
            nc.sync.dma_start(out=rmat_sb[:], in_=rmat_d)
            wsc_sb = tabp.tile([P, NB * 9], f32, tag="wsc")
            nc.sync.dma_start(out=wsc_sb[:], in_=wsc_d)
            bl_sb = tabp.tile([P, NB], f32, tag="bl")
            nc.sync.dma_start(out=bl_sb[:], in_=bl_d)
            br95_sb = tabp.tile([P, NB], f32, tag="br95")
            nc.sync.dma_start(out=br95_sb[:], in_=br95_d)
            br05_sb = tabp.tile([P, NB], f32, tag="br05")
            nc.sync.dma_start(out=br05_sb[:], in_=br05_d)

            # --- persistent image buffers ---
            A = bigp.tile([P, SZ], bf16, tag="A")   # band of x (residual source)
            B = bigp.tile([P, SZ], bf16, tag="B")
            D = bigp.tile([P, SZ], bf16, tag="D")
            for t in (A, B, D):
                nc.gpsimd.memset(t[:], 0.0)

            def v2d(t, row0, nrows, col0, ncols):
                return (t[:, 1:1 + S * WPAD]
                        .rearrange("p (r w) -> p r w", w=WPAD)
                        [:, row0:row0 + nrows, col0:col0 + ncols])

            def lhsT(s, t):
                i = (s * NMAT + t) * P
                return rmat_sb[:, i:i + P]

            for bi in range(n_bands):
                out_lo = bi * r_band
                out_hi = min(h, out_lo + r_band)
                in_lo = out_lo - 6

                # zero halo rows (6 top + 6 bottom of span)
                for t in (A, B, D):
                    nc.gpsimd.memset(t[:, 1:1 + 6 * WPAD], 0.0)
                    nc.gpsimd.memset(t[:, 1 + (S - 6) * WPAD:1 + S * WPAD], 0.0)

                # load x band (f32 -> bf16 cast DMA)
                img_lo = max(0, in_lo)
                img_hi = min(h, in_lo + S)
                nrows_ld = img_hi - img_lo
                r0 = img_lo - in_lo
                n_dma = 4
                step = (nrows_ld + n_dma - 1) // n_dma
                for k in range(0, nrows_ld, step):
                    kk = min(step, nrows_ld - k)
                    src = (xs_d[:, (img_lo + k) * W_IMG:(img_lo + k + kk) * W_IMG]
                           .rearrange("p (r w) -> p r w", w=W_IMG))
                    nc.gpsimd.dma_start(out=v2d(A, r0 + k, kk, 1, W_IMG), in_=src)

                xbuf = [A, B, D, B, D, B]
                ybuf = [B, D, B, D, B, D]
                for s in range(NB):
                    X, Y = xbuf[s], ybuf[s]
                    c_lo = max(0, out_lo - 5 + s)
                    c_hi = min(h, out_hi + 5 - s)
                    nr = c_hi - c_lo
                    r_off = c_lo - in_lo
                    e0 = 1 + r_off * WPAD

                    for cr0 in range(0, nr, CH_ROWS):
                        cr = min(CH_ROWS, nr - cr0)
                        n = cr * WPAD
                        base = e0 + cr0 * WPAD
                        # sub-chunks of <=512 elems, PSUM-bank aligned
                        subs = []
                        so = 0
                        while so < n:
                            subs.append((so, min(512, n - so)))
                            so += 512

                        # ---- conv-R: 9 PE taps -> psumR ----
                        psR = pspR.tile([P, 2048], f32, tag="psR")
                        for so, sn in subs:
                            for t in range(9):
                                dy, dx = TAPS[t]
                                off = (dy - 1) * WPAD + (dx - 1)
                                nc.tensor.matmul(
                                    psR[:, so:so + sn], lhsT(s, t),
                                    X[:, base + off + so:base + off + so + sn],
                                    start=(t == 0), stop=(t == 8))
                        # rR = 0.95*relu(vR+bR), uR = 0.05*(vR+bR)
                        rR = scrp.tile([P, CH_N], bf16, tag="rR")
                        nc.scalar.activation(
                            out=rR[:, :n], in_=psR[:, :n], func=relu,
                            bias=br95_sb[:, s:s + 1], scale=0.95)
                        uR = scrp.tile([P, CH_N], bf16, tag="uR")
                        nc.scalar.activation(
                            out=uR[:, :n], in_=psR[:, :n], func=ident,
                            bias=br05_sb[:, s:s + 1], scale=0.05)

                        # ---- conv-L: 4 PE taps -> psumL ----
                        psL = pspL.tile([P, 2048], f32, tag="psL")
                        for so, sn in subs:
                            for j, t in enumerate(L_PE_TAPS):
                                dy, dx = TAPS[t]
                                off = (dy - 1) * WPAD + (dx - 1)
                                nc.tensor.matmul(
                                    psL[:, so:so + sn], lhsT(s, 9 + j),
                                    X[:, base + off + so:base + off + so + sn],
                                    start=(j == 0), stop=(j == len(L_PE_TAPS) - 1))
                        cL = scrp.tile([P, CH_N], bf16, tag="cL")
                        nc.scalar.activation(
                            out=cL[:, :n], in_=psL[:, :n], func=ident,
                            bias=0.0, scale=0.05)

                        # ---- conv-L: 5 DVE taps into Y chunk (+bias) ----
                        ysl = v2d(Y, r_off + cr0, cr, 1, W_IMG)

                        def xv(t):
                            dy, dx = TAPS[t]
                            return v2d(X, r_off + cr0 + dy - 1, cr, dx, W_IMG)

                        t0 = L_DVE_TAPS[0]
                        nc.vector.tensor_scalar(
                            out=ysl, in0=xv(t0),
                            scalar1=wsc_sb[:, s * 9 + t0:s * 9 + t0 + 1],
                            scalar2=bl_sb[:, s:s + 1],
                            op0=AluOpType.mult, op1=AluOpType.add)
                        for t in L_DVE_TAPS[1:]:
                            tmp = scrp.tile([P, CH_N], bf16, tag="rL")
                            tv = (tmp[:, :n].rearrange("p (r w) -> p r w", w=WPAD)
                                  [:, :, 1:1 + W_IMG])
                            nc.vector.tensor_scalar(
                                out=tv, in0=xv(t),
                                scalar1=wsc_sb[:, s * 9 + t:s * 9 + t + 1],
                                scalar2=0.0,
                                op0=AluOpType.mult, op1=AluOpType.add)
                            nc.vector.tensor_tensor(
                                out=ysl, in0=ysl, in1=tv, op=AluOpType.add)
                        # += PE part of conv-L
                        cLv = (cL[:, :n].rearrange("p (r w) -> p r w", w=WPAD)
                               [:, :, 1:1 + W_IMG])
                        nc.vector.tensor_tensor(
                            out=ysl, in0=ysl, in1=cLv, op=AluOpType.add)

                        # ---- leaky-L + combine ----
                        # rL = 0.95*relu(accL) (ACT), Y = 0.05*accL + rL + rR + uR
                        rL = scrp.tile([P, CH_N], bf16, tag="rL")
                        rLv = (rL[:, :n].rearrange("p (r w) -> p r w", w=WPAD)
                               [:, :, 1:1 + W_IMG])
                        nc.scalar.activation(
                            out=rLv, in_=ysl, func=relu, bias=0.0, scale=19.0)
                        nc.vector.tensor_tensor(
                            out=ysl, in0=ysl, in1=rLv, op=AluOpType.add)
                        rRv = (rR[:, :n].rearrange("p (r w) -> p r w", w=WPAD)
                               [:, :, 1:1 + W_IMG])
                        nc.vector.tensor_tensor(
                            out=ysl, in0=ysl, in1=rRv, op=AluOpType.add)
                        uRv = (uR[:, :n].rearrange("p (r w) -> p r w", w=WPAD)
                               [:, :, 1:1 + W_IMG])
                        nc.vector.tensor_tensor(
                            out=ysl, in0=ysl, in1=uRv, op=AluOpType.add)

                # ---- residual + store (T6 = ybuf[5] = D) ----
                rows_per = 4
                for k in range(out_lo, out_hi, rows_per):
                    kk = min(rows_per, out_hi - k)
                    st = stgp.tile([P, rows_per * W_IMG], f32, tag="st")
                    stv = (st[:, :kk * W_IMG]
                           .rearrange("p (r w) -> p r w", w=W_IMG))
                    nc.vector.tensor_tensor(
                        out=stv, in0=v2d(D, k - in_lo, kk, 1, W_IMG),
                        in1=v2d(A, k - in_lo, kk, 1, W_IMG), op=AluOpType.add)
                    nc.sync.dma_start(
                        out=out_d[:, k * W_IMG:(k + kk) * W_IMG],
                        in_=st[:, :kk * W_IMG])

    nc.compile()
    return nc


def _host_tables(W, b, masks):
    """Build device-layout weight tables from full inputs."""
    Wt = np.asarray(W, np.float32).reshape(2 * NB, C, 3, 3)
    bt = np.asarray(b, np.float32)
    masks = np.asarray(masks, np.int64)

    # PE tap matrices: [P, NB*NMAT*P] bf16; lhsT[src, dst] per (stage, tap)
    rmat = np.zeros((NB, NMAT, P, P), np.float32)
    dst_c = np.arange(C)
    for s in range(NB):
        src_c = masks[s]
        for t in range(9):      # conv-R taps (gathered)
            dy, dx = TAPS[t]
            vals = Wt[2 * s + 1, dst_c, dy, dx]
            for bb in range(BC):
                rmat[s, t, bb * C + src_c, bb * C + dst_c] = vals
        for j, t in enumerate(L_PE_TAPS):   # conv-L PE taps (identity)
            dy, dx = TAPS[t]
            vals = Wt[2 * s, dst_c, dy, dx]
            for bb in range(BC):
                rmat[s, 9 + j, bb * C + dst_c, bb * C + dst_c] = vals
    rmat_sb = np.ascontiguousarray(
        rmat.transpose(2, 0, 1, 3).reshape(P, NB * NMAT * P))
    rmat_sb = rmat_sb.astype(ml_dtypes.bfloat16)

    # conv-L DVE weights and bias pre-scaled by 0.05: accL is accumulated
    # as 0.05*accL so leaky(accL) = relu(19*accL') + accL' needs no extra op
    pc = np.tile(np.arange(C), BC)
    wsc = np.zeros((P, NB * 9), np.float32)
    bl = np.zeros((P, NB), np.float32)
    br = np.zeros((P, NB), np.float32)
    for s in range(NB):
        for t in range(9):
            wsc[:, s * 9 + t] = 0.05 * Wt[2 * s, pc, t // 3, t % 3]
        bl[:, s] = 0.05 * bt[2 * s, pc]
        br[:, s] = bt[2 * s + 1, pc]
    return rmat_sb, wsc, bl, 0.95 * br, 0.05 * br


def _get_prog():
    key = (H, 96)
    if key not in _PROG_CACHE:
        _PROG_CACHE[key] = _build_program(H, 96)
    return _PROG_CACHE[key]


def _run_on_hw(nc, in_maps, trace=False, **kw):
    from concourse import bass_utils
    return bass_utils.run_bass_kernel_spmd(
        nc, in_maps, core_ids=list(range(len(in_maps))), trace=trace, **kw)


def _make_in_maps(x, W, b, masks):
    rmat_sb, wsc, bl, br95, br05 = _host_tables(W, b, masks)
    x = np.asarray(x, np.float32)
    nb_total = x.shape[0]
    in_maps = []
    for k in range(0, nb_total, BC):
        xs = np.ascontiguousarray(x[k:k + BC].reshape(BC * C, H * W_IMG))
        in_maps.append({"xs": xs, "rmat": rmat_sb, "wsc": wsc,
                       "bl": bl, "br95": br95, "br05": br05})
    return in_maps


def kernel(x, W, b, masks):
    nc = _get_prog()
    in_maps = _make_in_maps(x, W, b, masks)
    res = _run_on_hw(nc, in_maps)
    outs = [r["out"].reshape(BC, C, H, W_IMG).astype(np.float32)
            for r in res.results]
    return np.concatenate(outs, axis=0)


# revision 12
# speedup vs baseline: 1.5967x; 1.0707x over previous
"""ButterflyConv Trainium2 kernel.

Reference computation (per batch image):
  now = x
  for s in 0..5:
    left  = leaky(dwconv3x3(now,           W[2s])   + b[2s])
    right = leaky(dwconv3x3(now[masks[s]], W[2s+1]) + b[2s+1])
    now = left + right
  out = now + x
with leaky = LeakyReLU(0.05), SAME padding, depthwise (per-channel) 3x3 convs.

Mapping (per NeuronCore, 2 of 16 batch images):
  - 128 SBUF partitions = (batch 2) x (channel 64).
  - Free dim = image rows, padded to 194 cols (1 zero col each side) so
    horizontal conv shifts are plain element offsets.
  - H=192 in 2 bands of 96 output rows with a 6-row halo (6 chained 3x3
    stages shrink the valid region 1 row/stage).
  - conv-R (gathered branch, 9 taps) + 4 taps of conv-L run on the
    TensorEngine as PSUM-accumulated [128,128] matmuls whose lhsT holds
    W[conv, dst_c, dy, dx] at (src_p, dst_p); the channel gather is
    folded into the matrices. ~0.42ns/elem/tap.
  - Remaining 5 conv-L taps on VectorE as tensor_scalar (4x) +
    tensor_tensor (2x) pairs with per-partition weight scalars.
  - leaky(v+b) is decomposed as relu(0.95v + 0.95b) + (0.05v + 0.05b):
    both pieces are single ScalarE activation passes (Relu / Identity
    with per-partition bias, reading PSUM directly).
  - Combine adds are split across VectorE and GpSimd.
Data is bf16 on-chip; matmul accumulation is fp32 in PSUM; the final
residual add is computed in fp32.
"""

import numpy as np
import ml_dtypes

C = 64
H = 192
W_IMG = 192
NB = 6
BC = 2          # batch per core
P = 128
NCORES = 16 // BC
WPAD = W_IMG + 2
NEG = 0.05

# tap order: (dy, dx); first 9 = conv-R on PE; next 4 = conv-L PE taps
TAPS = [(dy, dx) for dy in range(3) for dx in range(3)]
L_PE_TAPS = [5, 6, 7, 8]          # conv-L taps done on PE
L_DVE_TAPS = [0, 1, 2, 3, 4]      # conv-L taps done on DVE
NMAT = 9 + len(L_PE_TAPS)         # matrices per stage
CH_ROWS = 10                      # epilogue chunk rows
CH_N = CH_ROWS * WPAD             # 1940 elems, fits 4 PSUM banks (2048)

_PROG_CACHE = {}


def _build_program(h=H, r_band=96, use_prelu=True):
    import concourse.bacc as bacc
    import concourse.mybir as mybir
    from concourse.tile import TileContext
    from concourse.alu_op_type import AluOpType

    f32 = mybir.dt.float32
    bf16 = mybir.dt.bfloat16
    ident = mybir.ActivationFunctionType.Identity
    relu = mybir.ActivationFunctionType.Relu
    prelu = mybir.ActivationFunctionType.Prelu

    S = r_band + 12               # band span rows
    SZ = S * WPAD + 2             # tile free size (+2 guard elems)
    n_bands = (h + r_band - 1) // r_band

    nc = bacc.Bacc("TRN2", target_bir_lowering=False, debug=False,
                   enable_asserts=False, num_devices=1)

    xs_d = nc.dram_tensor("xs", [P, h * W_IMG], f32, kind="ExternalInput").ap()
    rmat_d = nc.dram_tensor("rmat", [P, NB * NMAT * P], bf16,
                            kind="ExternalInput").ap()
    wsc_d = nc.dram_tensor("wsc", [P, NB * 9], f32, kind="ExternalInput").ap()
    bl_d = nc.dram_tensor("bl", [P, NB], f32, kind="ExternalInput").ap()
    # right-branch biases pre-scaled by 0.95 / 0.05 on host
    br95_d = nc.dram_tensor("br95", [P, NB], f32, kind="ExternalInput").ap()
    br05_d = nc.dram_tensor("br05", [P, NB], f32, kind="ExternalInput").ap()
    br_d = nc.dram_tensor("br", [P, NB], f32, kind="ExternalInput").ap()
    out_d = nc.dram_tensor("out", [P, h * W_IMG], f32, kind="ExternalOutput").ap()

    with TileContext(nc) as tc:
        with tc.tile_pool(name="big", bufs=1) as bigp, \
             tc.tile_pool(name="tab", bufs=1) as tabp, \
             tc.tile_pool(name="scr", bufs=3) as scrp, \
             tc.tile_pool(name="stg", bufs=3) as stgp, \
             tc.tile_pool(name="psR", bufs=1, space="PSUM") as pspR, \
             tc.tile_pool(name="psL", bufs=1, space="PSUM") as pspL:

            # --- static tables ---
            rmat_sb = tabp.tile([P, NB * NMAT * P], bf16, tag="rmat")
            nc.sync.dma_start(out=rmat_sb[:], in_=rmat_d)
            wsc_sb = tabp.tile([P, NB * 9], f32, tag="wsc")
            nc.sync.dma_start(out=wsc_sb[:], in_=wsc_d)
            bl_sb = tabp.tile([P, NB], f32, tag="bl")
            nc.sync.dma_start(out=bl_sb[:], in_=bl_d)
            br95_sb = tabp.tile([P, NB], f32, tag="br95")
            nc.sync.dma_start(out=br95_sb[:], in_=br95_d)
            br05_sb = tabp.tile([P, NB], f32, tag="br05")
            nc.sync.dma_start(out=br05_sb[:], in_=br05_d)
            br_sb = tabp.tile([P, NB], f32, tag="br")
            nc.sync.dma_start(out=br_sb[:], in_=br_d)

            # --- persistent image buffers ---
            A = bigp.tile([P, SZ], bf16, tag="A")   # band of x (residual source)
            B = bigp.tile([P, SZ], bf16, tag="B")
            D = bigp.tile([P, SZ], bf16, tag="D")
            for t in (A, B, D):
                nc.gpsimd.memset(t[:], 0.0)

            def v2d(t, row0, nrows, col0, ncols):
                return (t[:, 1:1 + S * WPAD]
                        .rearrange("p (r w) -> p r w", w=WPAD)
                        [:, row0:row0 + nrows, col0:col0 + ncols])

            def lhsT(s, t):
                i = (s * NMAT + t) * P
                return rmat_sb[:, i:i + P]

            for bi in range(n_bands):
                out_lo = bi * r_band
                out_hi = min(h, out_lo + r_band)
                in_lo = out_lo - 6

                # zero halo rows (6 top + 6 bottom of span)
                for t in (A, B, D):
                    nc.gpsimd.memset(t[:, 1:1 + 6 * WPAD], 0.0)
                    nc.gpsimd.memset(t[:, 1 + (S - 6) * WPAD:1 + S * WPAD], 0.0)

                # load x band (f32 -> bf16 cast DMA)
                img_lo = max(0, in_lo)
                img_hi = min(h, in_lo + S)
                nrows_ld = img_hi - img_lo
                r0 = img_lo - in_lo
                n_dma = 4
                step = (nrows_ld + n_dma - 1) // n_dma
                for k in range(0, nrows_ld, step):
                    kk = min(step, nrows_ld - k)
                    src = (xs_d[:, (img_lo + k) * W_IMG:(img_lo + k + kk) * W_IMG]
                           .rearrange("p (r w) -> p r w", w=W_IMG))
                    nc.gpsimd.dma_start(out=v2d(A, r0 + k, kk, 1, W_IMG), in_=src)

                xbuf = [A, B, D, B, D, B]
                ybuf = [B, D, B, D, B, D]
                for s in range(NB):
                    X, Y = xbuf[s], ybuf[s]
                    c_lo = max(0, out_lo - 5 + s)
                    c_hi = min(h, out_hi + 5 - s)
                    nr = c_hi - c_lo
                    r_off = c_lo - in_lo
                    e0 = 1 + r_off * WPAD

                    for cr0 in range(0, nr, CH_ROWS):
                        cr = min(CH_ROWS, nr - cr0)
                        n = cr * WPAD
                        base = e0 + cr0 * WPAD
                        # sub-chunks of <=512 elems, PSUM-bank aligned
                        subs = []
                        so = 0
                        while so < n:
                            subs.append((so, min(512, n - so)))
                            so += 512

                        # ---- conv-R: 9 PE taps -> psumR ----
                        psR = pspR.tile([P, 2048], f32, tag="psR")
                        for so, sn in subs:
                            for t in range(9):
                                dy, dx = TAPS[t]
                                off = (dy - 1) * WPAD + (dx - 1)
                                nc.tensor.matmul(
                                    psR[:, so:so + sn], lhsT(s, t),
                                    X[:, base + off + so:base + off + so + sn],
                                    start=(t == 0), stop=(t == 8))
                        rR = scrp.tile([P, CH_N], bf16, tag="rR")
                        if use_prelu:
                            # rR = leaky(vR + bR) in one pass
                            nc.scalar.activation(
                                out=rR[:, :n], in_=psR[:, :n], func=prelu,
                                bias=br_sb[:, s:s + 1], scale=1.0, alpha=NEG)
                        else:
                            # rR = 0.95*relu(vR+bR); uR = 0.05*(vR+bR)
                            nc.scalar.activation(
                                out=rR[:, :n], in_=psR[:, :n], func=relu,
                                bias=br95_sb[:, s:s + 1], scale=0.95)
                            uR = scrp.tile([P, CH_N], bf16, tag="uR")
                            nc.scalar.activation(
                                out=uR[:, :n], in_=psR[:, :n], func=ident,
                                bias=br05_sb[:, s:s + 1], scale=0.05)

                        # ---- conv-L: 4 PE taps -> psumL ----
                        psL = pspL.tile([P, 2048], f32, tag="psL")
                        for so, sn in subs:
                            for j, t in enumerate(L_PE_TAPS):
                                dy, dx = TAPS[t]
                                off = (dy - 1) * WPAD + (dx - 1)
                                nc.tensor.matmul(
                                    psL[:, so:so + sn], lhsT(s, 9 + j),
                                    X[:, base + off + so:base + off + so + sn],
                                    start=(j == 0), stop=(j == len(L_PE_TAPS) - 1))
                        cL = scrp.tile([P, CH_N], bf16, tag="cL")
                        nc.scalar.activation(
                            out=cL[:, :n], in_=psL[:, :n], func=ident,
                            bias=0.0, scale=0.05)

                        # ---- conv-L: 5 DVE taps into Y chunk (+bias) ----
                        ysl = v2d(Y, r_off + cr0, cr, 1, W_IMG)

                        def xv(t):
                            dy, dx = TAPS[t]
                            return v2d(X, r_off + cr0 + dy - 1, cr, dx, W_IMG)

                        t0 = L_DVE_TAPS[0]
                        nc.vector.tensor_scalar(
                            out=ysl, in0=xv(t0),
                            scalar1=wsc_sb[:, s * 9 + t0:s * 9 + t0 + 1],
                            scalar2=bl_sb[:, s:s + 1],
                            op0=AluOpType.mult, op1=AluOpType.add)
                        for t in L_DVE_TAPS[1:]:
                            tmp = scrp.tile([P, CH_N], bf16, tag="rL")
                            tv = (tmp[:, :n].rearrange("p (r w) -> p r w", w=WPAD)
                                  [:, :, 1:1 + W_IMG])
                            nc.vector.tensor_scalar(
                                out=tv, in0=xv(t),
                                scalar1=wsc_sb[:, s * 9 + t:s * 9 + t + 1],
                                scalar2=0.0,
                                op0=AluOpType.mult, op1=AluOpType.add)
                            nc.vector.tensor_tensor(
                                out=ysl, in0=ysl, in1=tv, op=AluOpType.add)
                        # += PE part of conv-L
                        cLv = (cL[:, :n].rearrange("p (r w) -> p r w", w=WPAD)
                               [:, :, 1:1 + W_IMG])
                        nc.vector.tensor_tensor(
                            out=ysl, in0=ysl, in1=cLv, op=AluOpType.add)

                        # ---- leaky-L + combine ----
                        rL = scrp.tile([P, CH_N], bf16, tag="rL")
                        rLv = (rL[:, :n].rearrange("p (r w) -> p r w", w=WPAD)
                               [:, :, 1:1 + W_IMG])
                        rRv = (rR[:, :n].rearrange("p (r w) -> p r w", w=WPAD)
                               [:, :, 1:1 + W_IMG])
                        if use_prelu:
                            # accL is 0.05-scaled; leaky(accL) = leaky(20*accL')
                            nc.scalar.activation(
                                out=rLv, in_=ysl, func=prelu,
                                bias=0.0, scale=20.0, alpha=NEG)
                            nc.vector.tensor_tensor(
                                out=ysl, in0=rLv, in1=rRv, op=AluOpType.add)
                        else:
                            # rL = 0.95*relu(accL); Y = 0.05*accL + rL + rR + uR
                            nc.scalar.activation(
                                out=rLv, in_=ysl, func=relu, bias=0.0, scale=19.0)
                            nc.vector.tensor_tensor(
                                out=ysl, in0=ysl, in1=rLv, op=AluOpType.add)
                            nc.vector.tensor_tensor(
                                out=ysl, in0=ysl, in1=rRv, op=AluOpType.add)
                            uRv = (uR[:, :n].rearrange("p (r w) -> p r w", w=WPAD)
                                   [:, :, 1:1 + W_IMG])
                            nc.vector.tensor_tensor(
                                out=ysl, in0=ysl, in1=uRv, op=AluOpType.add)

                # ---- residual + store (T6 = ybuf[5] = D) ----
                rows_per = 4
                for k in range(out_lo, out_hi, rows_per):
                    kk = min(rows_per, out_hi - k)
                    st = stgp.tile([P, rows_per * W_IMG], f32, tag="st")
                    stv = (st[:, :kk * W_IMG]
                           .rearrange("p (r w) -> p r w", w=W_IMG))
                    nc.vector.tensor_tensor(
                        out=stv, in0=v2d(D, k - in_lo, kk, 1, W_IMG),
                        in1=v2d(A, k - in_lo, kk, 1, W_IMG), op=AluOpType.add)
                    nc.sync.dma_start(
                        out=out_d[:, k * W_IMG:(k + kk) * W_IMG],
                        in_=st[:, :kk * W_IMG])

    nc.compile()
    return nc


def _host_tables(W, b, masks):
    """Build device-layout weight tables from full inputs."""
    Wt = np.asarray(W, np.float32).reshape(2 * NB, C, 3, 3)
    bt = np.asarray(b, np.float32)
    masks = np.asarray(masks, np.int64)

    # PE tap matrices: [P, NB*NMAT*P] bf16; lhsT[src, dst] per (stage, tap)
    rmat = np.zeros((NB, NMAT, P, P), np.float32)
    dst_c = np.arange(C)
    for s in range(NB):
        src_c = masks[s]
        for t in range(9):      # conv-R taps (gathered)
            dy, dx = TAPS[t]
            vals = Wt[2 * s + 1, dst_c, dy, dx]
            for bb in range(BC):
                rmat[s, t, bb * C + src_c, bb * C + dst_c] = vals
        for j, t in enumerate(L_PE_TAPS):   # conv-L PE taps (identity)
            dy, dx = TAPS[t]
            vals = Wt[2 * s, dst_c, dy, dx]
            for bb in range(BC):
                rmat[s, 9 + j, bb * C + dst_c, bb * C + dst_c] = vals
    rmat_sb = np.ascontiguousarray(
        rmat.transpose(2, 0, 1, 3).reshape(P, NB * NMAT * P))
    rmat_sb = rmat_sb.astype(ml_dtypes.bfloat16)

    # conv-L DVE weights and bias pre-scaled by 0.05: accL is accumulated
    # as 0.05*accL so leaky(accL) = relu(19*accL') + accL' needs no extra op
    pc = np.tile(np.arange(C), BC)
    wsc = np.zeros((P, NB * 9), np.float32)
    bl = np.zeros((P, NB), np.float32)
    br = np.zeros((P, NB), np.float32)
    for s in range(NB):
        for t in range(9):
            wsc[:, s * 9 + t] = 0.05 * Wt[2 * s, pc, t // 3, t % 3]
        bl[:, s] = 0.05 * bt[2 * s, pc]
        br[:, s] = bt[2 * s + 1, pc]
    return rmat_sb, wsc, bl, br, 0.95 * br, 0.05 * br


USE_PRELU = True


def _get_prog():
    key = (H, 96, USE_PRELU)
    if key not in _PROG_CACHE:
        _PROG_CACHE[key] = _build_program(H, 96, use_prelu=USE_PRELU)
    return _PROG_CACHE[key]


def _run_on_hw(nc, in_maps, trace=False, **kw):
    from concourse import bass_utils
    return bass_utils.run_bass_kernel_spmd(
        nc, in_maps, core_ids=list(range(len(in_maps))), trace=trace, **kw)


def _make_in_maps(x, W, b, masks):
    rmat_sb, wsc, bl, br, br95, br05 = _host_tables(W, b, masks)
    x = np.asarray(x, np.float32)
    nb_total = x.shape[0]
    in_maps = []
    for k in range(0, nb_total, BC):
        xs = np.ascontiguousarray(x[k:k + BC].reshape(BC * C, H * W_IMG))
        in_maps.append({"xs": xs, "rmat": rmat_sb, "wsc": wsc, "bl": bl,
                       "br": br, "br95": br95, "br05": br05})
    return in_maps


def kernel(x, W, b, masks):
    nc = _get_prog()
    in_maps = _make_in_maps(x, W, b, masks)
    res = _run_on_hw(nc, in_maps)
    outs = [r["out"].reshape(BC, C, H, W_IMG).astype(np.float32)
            for r in res.results]
    return np.concatenate(outs, axis=0)
